# revision 43
# baseline (speedup 1.0000x reference)
"""GQA kernel for Trainium2, 8 NeuronCores — v2 (no collectives).

Problem: B=2, T=2048, HIDDEN=1024, 16 q-heads, 4 kv-heads, head_dim=64,
causal attention + output projection.

Sharding: core = (batch b = core//4, kv-group g = core%4). Each core handles
one batch element and the 4 query heads sharing kv-head g. o_proj is
ROW-parallel: each core multiplies its local 256 attention dims by the
matching 256 rows of Wo, producing a full [1024, 2048] f32 partial that the
HOST sums across the 4 cores of each batch group (no device collective).

Device dataflow (all matmuls bf16 with fp32 PSUM accumulation):
  - host supplies xT = x[b].T in bf16 ([1024, 2048]; hidden on partitions)
  - qT/kT via W-stationary matmuls (outputs transposed: head_dim on
    partitions); V natural via PE transposes of vT tiles; ones column
    appended so softmax denominators fall out of the PV matmul for free
  - S^T = kT.T @ qT directly; 2 heads packed per pass via PE row-tiling
    (K=64 each); causal-diagonal tiles trim their fully-masked q columns
    from the QK / EXP / PV work; [128,128] triangle mask on the partial
    block only
  - exp on ACT engine; PV software-pipelined 2 units behind QK so the ACT
    engine (the attention-phase bottleneck) never starves the PE queue
  - o_proj of chunk c-1 and qkv projection of chunk c+1 are interleaved
    into chunk c's attention to fill PE idle slots
  - normalization reads PSUM directly: fast-approx reciprocal of the
    denominator row, gpsimd partition-broadcast, one DVE multiply per head
"""

import sys

import numpy as np

try:
    import concourse.bass as bass
except ImportError:
    sys.path.insert(0, "/opt/trn_rl_repo")
    import concourse.bass as bass

import ml_dtypes
from contextlib import ExitStack

import concourse.tile as tile
from concourse import bacc, mybir
from concourse.bass import ds, ts
from concourse.bass_utils import run_bass_kernel_spmd
from concourse.masks import make_identity

BF16 = mybir.dt.bfloat16
F32 = mybir.dt.float32

P = 128
T = 2048
HID = 1024
KT = HID // P  # 8 k-tiles over hidden
CH = 512       # T_q chunk width
NCHUNK = T // CH
D = 64         # head dim
SCALE = D ** -0.5

_PROGRAM = None


def build_program():
    nc = bacc.Bacc(num_devices=8)

    xT_d = nc.declare_dram_parameter("xT", [HID, T], BF16, isOutput=False)
    wqkv_d = nc.declare_dram_parameter("wqkv", [HID, 384], BF16, isOutput=False)
    wo_d = nc.declare_dram_parameter("wo", [P, 2 * HID], BF16, isOutput=False)
    mask_d = nc.declare_dram_parameter("maskt", [P, P], BF16, isOutput=False)
    outT_d = nc.declare_dram_parameter("outT", [HID, T], F32, isOutput=True)
    # kt1 (heads 2,3) partial of the final chunk's o_proj — host adds it
    # into outT[:, -CH:]; lets kt0 o_proj start before heads 2,3 normalize
    outT1_d = nc.declare_dram_parameter("outT1", [HID, CH], BF16, isOutput=True)

    with tile.TileContext(nc) as tc, ExitStack() as ctx:
        sing = ctx.enter_context(tc.tile_pool(name="sing", bufs=1))
        # hp-major unit order keeps only 2 attention accumulators live at a
        # time, so accp needs 2 banks and the score ring can go 3 deep
        work = ctx.enter_context(tc.tile_pool(name="work", bufs=3, space="PSUM"))
        accp = ctx.enter_context(tc.tile_pool(name="accp", bufs=2, space="PSUM"))
        ptp = ctx.enter_context(tc.tile_pool(name="ptp", bufs=6))
        nrmp = ctx.enter_context(tc.tile_pool(name="nrmp", bufs=8))
        outp = ctx.enter_context(tc.tile_pool(name="outp", bufs=3))
        agp = ctx.enter_context(tc.tile_pool(name="agp", bufs=2))

        # --- loads needed before chunk-0 compute, kt-interleaved ---
        wqkv_sb = sing.tile([P, KT, 384], BF16)
        xT_sb = sing.tile([P, KT, T], BF16)
        # trigger-issue latency (~0.6us/DMA) gates the first matmul: split
        # the chunk-0-critical loads across both HWDGE engines (SP + ACT)
        for kt in range(KT):
            eng = nc.scalar if kt % 2 else nc.sync
            eng.dma_start(wqkv_sb[:, kt, :], wqkv_d[ts(kt, P), :])
            eng.dma_start(xT_sb[:, kt, ts(0, CH)], xT_d[ts(kt, P), ts(0, CH)])
        maskt = sing.tile([P, P], BF16)
        nc.sync.dma_start(maskt, mask_d[:, :])
        ident = sing.tile([P, P], BF16)
        make_identity(nc, ident)
        # --- warmup during the initial DMA wait: preload the ACT exp table
        # (~2.7us) and run the PE busy so HAM unthrottles to 2.4 GHz ---
        wrm = sing.tile([1, 4], F32)
        nc.gpsimd.memset(wrm, 0.0)
        nc.scalar.activation(wrm, wrm, mybir.ActivationFunctionType.Exp)
        wps = work.tile([P, P], F32, tag="work", name="warm_ps")
        for _ in range(48):
            nc.tensor.matmul(wps, ident, ident, start=True, stop=True)
        # --- deferred loads ---
        for c in range(1, NCHUNK):
            for kt in range(KT):
                nc.sync.dma_start(xT_sb[:, kt, ts(c, CH)], xT_d[ts(kt, P), ts(c, CH)])
        wo_sb = sing.tile([P, 2, HID], BF16)
        nc.sync.dma_start(wo_sb, wo_d[:, :].rearrange("p (kt n) -> p kt n", kt=2))

        # blocks: 0 = qT heads (0,1); 1 = qT heads (2,3); 2 = [kT | vT]
        qkvT_sb = sing.tile([P, 3, T], BF16)
        kdup = sing.tile([P, T], BF16)        # kT duplicated on both partition halves
        vaug = sing.tile([P, 16, 66], BF16)   # V natural per T_k tile + ones col (64)
        nc.gpsimd.memset(vaug[:, :, 64:65], 1.0)

        def emit_qkv_proj(c, blks=(2, 0, 1)):
            # blk2 (k|v) first so its DVE copies land early; q-block copies
            # go to ACT (idle at chunk boundaries) to unblock the next
            # chunk's QK matmuls without waiting on the DVE queue
            cs = ts(c, CH)
            for blk in blks:
                pj = work.tile([P, CH], F32, tag="work", name=f"pj{c}_{blk}")
                for kt in range(KT):
                    nc.tensor.matmul(
                        pj,
                        wqkv_sb[:, kt, ts(blk, P)],
                        xT_sb[:, kt, cs],
                        start=(kt == 0),
                        stop=(kt == KT - 1),
                    )
                if blk == 0:
                    # heads 0,1: every hp0 unit of the next chunk reads this
                    # — ACT (idle at the boundary) delivers it fastest
                    nc.scalar.copy(qkvT_sb[:, blk, cs], pj)
                elif blk == 1:
                    # heads 2,3: not read until the hp-switch — DVE has time
                    nc.vector.tensor_copy(qkvT_sb[:, blk, cs], pj)
                else:
                    nc.vector.tensor_copy(kdup[0:64, cs], pj[0:64, :])
                    nc.vector.tensor_copy(kdup[64:128, cs], pj[0:64, :])
                    nc.vector.tensor_copy(qkvT_sb[64:128, 2, cs], pj[64:128, :])

        def emit_vT(c):
            for j in range(4 * c, 4 * c + 4):
                vps = work.tile([P, 64], BF16, tag="work", name=f"vps{j}")
                nc.tensor.transpose(
                    vps[:, 0:64], qkvT_sb[64:128, 2, ts(j, P)], ident[64:128, 64:128]
                )
                nc.vector.tensor_copy(vaug[:, j, 0:64], vps[:, 0:64])

        def emit_oproj_item(c, mb):
            # one m-tile of the row-parallel o_proj partial for chunk c
            at = atst[c]
            ps = work.tile([P, CH], F32, tag="work", name=f"ps{c}_{mb}")
            for kt in range(2):
                nc.tensor.matmul(
                    ps,
                    wo_sb[:, kt, ts(mb, P)],
                    at[:, kt, :],
                    start=(kt == 0),
                    stop=(kt == 1),
                )
            ob = outp.tile([P, CH], F32, tag="ob", name=f"ob{c}_{mb}")
            nc.vector.tensor_copy(ob, ps)
            nc.sync.dma_start(outT_d[ts(mb, P), ts(c, CH)], ob)

        def emit_oproj_kt(c, mb, kt):
            # final-chunk o_proj split by kt tile (head pair); kt1 goes to
            # the separate outT1 partial the host adds back in
            at = atst[c]
            ps = work.tile([P, CH], F32, tag="work", name=f"psk{c}_{mb}_{kt}")
            nc.tensor.matmul(
                ps, wo_sb[:, kt, ts(mb, P)], at[:, kt, :], start=True, stop=True
            )
            if kt == 0:
                ob = outp.tile([P, CH], F32, tag="ob", name=f"obk{c}_{mb}_{kt}")
                nc.scalar.copy(ob, ps)  # ACT is idle in the final-chunk tail
                nc.sync.dma_start(outT_d[ts(mb, P), ts(c, CH)], ob)
            else:
                ob = outp.tile([P, CH], BF16, tag="ob1", name=f"obk{c}_{mb}_{kt}")
                nc.scalar.copy(ob, ps)
                nc.sync.dma_start(outT1_d[ts(mb, P), :], ob)

        def emit_normalize_pair(c, oa, at, hs):
            # normalize one head-pair; o_proj kt-tile MMs unblock per pair
            for h in hs:
                dn = nrmp.tile([1, CH], F32, tag="dn", name=f"dn{c}_{h}")
                nc.vector.tensor_copy(dn, oa[h][64:65, :])
                rc = nrmp.tile([1, CH], F32, tag="rc", name=f"rc{c}_{h}")
                nc.vector.reciprocal_approx_fast(out=rc, in_=dn)
                rb = nrmp.tile([64, CH], F32, tag="rb", name=f"rb{c}_{h}")
                nc.gpsimd.partition_broadcast(rb, rc)
                nc.vector.tensor_mul(
                    at[ds(64 * (h % 2), 64), h // 2, :], oa[h][0:64, :], rb
                )

        atst = {}

        # steady-state per-chunk loop with software pipelining
        emit_qkv_proj(0)
        emit_vT(0)
        for c in range(NCHUNK):
            ntk = 4 * (c + 1)
            final = c == NCHUNK - 1
            oa = [
                accp.tile([65, CH], F32, tag="acc", name=f"oa{c}_{h}")
                for h in range(2)
            ]
            at = agp.tile([P, 2, CH], BF16, tag="atst", name=f"atst{c}")
            atst[c] = at

            # hp-major: heads 0,1 finish first; their normalize frees the
            # accumulator banks for heads 2,3 and (final chunk) overlaps
            # the kt0 o_proj matmuls with the hp1 half of the chunk
            units = [(j, 0) for j in range(ntk)] + [(j, 1) for j in range(ntk)]
            pend = []

            def emit_pv(j, hp):
                r = j - 4 * c
                lo = 128 * r if r >= 0 else 0
                for hh in range(2):
                    h = 2 * hp + hh
                    nc.tensor.matmul(
                        oa[h][0:65, lo:CH],
                        vaug[:, j, 0:65],
                        pt_of[(j, hp)][:, ds(CH * hh + lo, CH - lo)],
                        start=(j == 0),
                        stop=(j == ntk - 1),
                    )
                if j == ntk - 1 and hp == 0:
                    emit_normalize_pair(c, oa, at, (0, 1))
                    for h in (2, 3):
                        oa.append(
                            accp.tile([65, CH], F32, tag="acc", name=f"oa{c}_{h}")
                        )
                    if final:
                        kt0_fills.extend(range(KT))

            # o_proj(c-1) m-tiles spread one-per-unit starting at unit 6
            fills = list(range(KT)) if c >= 1 else []
            vt_fills = list(range(4 * c, 4 * c + 4)) if c >= 1 else []
            kt0_fills = []

            pt_of = {}
            for i, (j, hp) in enumerate(units):
                r = j - 4 * c
                diag = r >= 0
                lo = 128 * r if diag else 0
                # ---- QK: scores^T for heads (2hp, 2hp+1), k-tile j ----
                s2 = work.tile([P, 1024], F32, tag="work", name=f"s2_{c}_{j}_{hp}")
                nc.tensor.matmul(
                    s2[:, lo:CH],
                    kdup[0:64, ts(j, P)],
                    qkvT_sb[0:64, hp, ds(CH * c + lo, CH - lo)],
                    start=True,
                    stop=True,
                    tile_position=(0, 0),
                )
                nc.tensor.matmul(
                    s2[:, CH + lo:1024],
                    kdup[64:128, ts(j, P)],
                    qkvT_sb[64:128, hp, ds(CH * c + lo, CH - lo)],
                    start=True,
                    stop=True,
                    tile_position=(64, 0),
                )
                # ---- exp on ACT (both heads' windows in one instr) ----
                pt = ptp.tile([P, 1024], BF16, tag="pt", name=f"pt{c}_{j}_{hp}")
                pt_of[(j, hp)] = pt
                if lo:
                    s2v = s2.rearrange("p (h q) -> p h q", h=2)[:, :, lo:CH]
                    ptv = pt.rearrange("p (h q) -> p h q", h=2)[:, :, lo:CH]
                    nc.scalar.activation(ptv, s2v, mybir.ActivationFunctionType.Exp)
                else:
                    nc.scalar.activation(pt, s2, mybir.ActivationFunctionType.Exp)
                if diag:
                    # triangle mask on the partial 128-col block of each head
                    nc.vector.tensor_mul(
                        pt[:, ds(lo, P)], pt[:, ds(lo, P)], maskt
                    )
                    nc.vector.tensor_mul(
                        pt[:, ds(CH + lo, P)], pt[:, ds(CH + lo, P)], maskt
                    )
                pend.append((j, hp))
                if c == 0 and i in (2, 4, 6):
                    # chunk 0 is thin on PE work: emit chunk 1's qkv blocks
                    # here; blk2's copies then precede the normalize chain
                    emit_qkv_proj(1, ((2, 0, 1)[i // 2 - 1],))
                if i >= 3 and vt_fills:
                    jv = vt_fills.pop(0)
                    vps = work.tile([P, 64], BF16, tag="work", name=f"vps{jv}")
                    nc.tensor.transpose(
                        vps[:, 0:64],
                        qkvT_sb[64:128, 2, ts(jv, P)],
                        ident[64:128, 64:128],
                    )
                    nc.vector.tensor_copy(vaug[:, jv, 0:64], vps[:, 0:64])
                if i >= 8 and i % 2 == 0 and fills:
                    emit_oproj_item(c - 1, fills.pop(0))
                if len(pend) > 2:
                    emit_pv(*pend.pop(0))
            while fills:
                emit_oproj_item(c - 1, fills.pop(0))

            # drain: interleave next chunk's qkv-proj (no EXP dependency)
            # with the trailing PVs so the PE never waits on the ACT queue;
            # the pj DVE copies land in the queue BEFORE the normalize chain
            if not final:
                if c > 0:
                    emit_qkv_proj(c + 1)
                emit_pv(*pend.pop(0))
                emit_pv(*pend.pop(0))
                emit_normalize_pair(c, oa, at, (2, 3))
            else:
                while pend:
                    emit_pv(*pend.pop(0))
                while kt0_fills:
                    emit_oproj_kt(c, kt0_fills.pop(0), 0)
                emit_normalize_pair(c, oa, at, (2, 3))
                for mb in range(KT):
                    emit_oproj_kt(c, mb, 1)

    nc.finalize()
    return nc


def _prep_inputs(x, Wq, Wkv, Wo):
    bf = ml_dtypes.bfloat16
    x = np.asarray(x, dtype=np.float32)
    Wq = np.asarray(Wq, dtype=np.float32)
    Wkv = np.asarray(Wkv, dtype=np.float32)
    Wo = np.asarray(Wo, dtype=np.float32)

    # triangle mask: M[p, m] = 1.0 iff p <= m (kpos-local <= q-local)
    mask = (np.arange(P)[:, None] <= np.arange(P)[None, :]).astype(bf)

    xT = [np.ascontiguousarray(x[b].T).astype(bf) for b in range(2)]

    in_maps = []
    for core in range(8):
        b, g = core // 4, core % 4
        wq_g = Wq[:, 256 * g : 256 * (g + 1)] * SCALE
        wk_g = Wkv[:, 64 * g : 64 * (g + 1)]
        wv_g = Wkv[:, 256 + 64 * g : 256 + 64 * (g + 1)]
        wqkv = np.ascontiguousarray(
            np.concatenate([wq_g, wk_g, wv_g], axis=1)
        ).astype(bf)
        # lhsT layout for row-parallel o_proj: [p, kt, m] = Wo[256g+128kt+p, m]
        wo_g = np.ascontiguousarray(
            Wo[256 * g : 256 * (g + 1), :]
            .reshape(2, P, HID)
            .transpose(1, 0, 2)
            .reshape(P, 2 * HID)
        ).astype(bf)
        in_maps.append(
            {"xT": xT[b], "wqkv": wqkv, "wo": wo_g, "maskt": mask}
        )
    return in_maps


def run(x, Wq, Wkv, Wo, trace=False, **trace_kwargs):
    global _PROGRAM
    if _PROGRAM is None:
        _PROGRAM = build_program()
    nc = _PROGRAM
    in_maps = _prep_inputs(x, Wq, Wkv, Wo)
    res = run_bass_kernel_spmd(
        nc, in_maps, core_ids=list(range(8)), trace=trace, **trace_kwargs
    )
    outs = res.results
    full = np.empty((2, T, HID), dtype=np.float32)
    for b in range(2):
        acc = np.zeros((HID, T), dtype=np.float32)
        for g in range(4):
            acc += np.asarray(outs[4 * b + g]["outT"])
            acc[:, T - CH :] += np.asarray(outs[4 * b + g]["outT1"]).astype(
                np.float32
            )
        full[b] = acc.T
    return full, res


def kernel(x, Wq, Wkv, Wo):
    out, _ = run(x, Wq, Wkv, Wo, trace=False)
    return out
